# revision 34
# baseline (speedup 1.0000x reference)
"""Multi-head attention kernel for 8 TRN2 NeuronCores.

Problem: b=2, n=2048, d=1024, heads=16, hd=64.
  q/k/v = x @ W{q,k,v}.T (+ zero bias)
  per head: softmax(q k^T / sqrt(d)) @ v
  out = concat @ Wo.T (+ zero bias)

Sharding (8 cores): data-parallel over batch (2) x tensor-parallel over
heads (16 heads -> 4 groups of 4). Core c handles batch c//4, heads
4*(c%4) .. 4*(c%4)+3 (feature slice of 256 columns). Wo is applied
row-parallel: each core emits a partial (n, d) output; the host sums the
4 partials per batch. No collectives needed.

All matmuls run in float32r (TF32-like: ~1.5e-4 rel err on a K=1024
contraction, 4x the fp32 rate). Operands feeding f32r matmuls must be
produced "rounded": DMA'd tensors get one DVE conversion pass; on-chip
tensors (Q^T/K^T/V/P^T/out^T) are written as f32r by their producing
copy/activation directly.

Per-core layouts (host pre-transposes so no on-device transposes at all):
  xT  (d, n)   : x[b].T
  wqT/wkT/wvT (d, 256), woT (256, d)
Pipeline:
  QT[feat, n], KT[feat, n]  (PE; contraction over d; f32r out via DVE)
  V[n, feat] + ones column  (PE; natural layout for AV stationary)
  per head h, k-chunk kc (128 k's), q-half sh (1024 q's):
     scores^T[128, 1024] = KT_h^T . QT_h   (PE, K=hd=64, psum)
     P^T = exp(scores^T / 32)              (ACT, psum->sbuf, f32r out)
     avo[65, q] += V_aug^T . P^T           (PE; row 64 = softmax sums)
  normalize: recip(sums) -> partition_broadcast -> mul  (DVE+GPSIMD)
  partial[n, d] = outT^T . woT (PE), DMA out via SBUF.

Biases are structurally zero in this problem spec and are skipped.
"""

import numpy as np

HEADS = 16
D = 1024
N = 2048
B = 2
N_CORES = 8
HPC = HEADS // (N_CORES // B)  # heads per core = 4
HD = D // HEADS                # 64
F = HPC * HD                   # 256 features per core
P = 128


def build_nc(n=N, d=D, hpc=HPC, hd=HD):
    """Build the per-core Bass program (SPMD: same program on all 8 cores)."""
    import concourse.bass as bass
    import concourse.tile as tile
    from concourse import bacc, mybir

    f32 = mybir.dt.float32
    f32r = mybir.dt.float32r
    f = hpc * hd            # per-core feature count (256)
    FC = f // P             # feature chunks (2)
    DC = d // P             # contraction chunks over d (8)
    NT = n // P             # n tiles / k chunks (16)
    QB = min(512, n)        # matmul moving block
    SCW = min(1024, n)      # scores psum width (2 banks)
    NSC = n // SCW          # q-halves
    scale = 1.0 / float(np.sqrt(np.float32(d)))

    nc = bacc.Bacc("TRN2")

    # inputs are declared float32r: the PE accepts raw fp32 bits via direct
    # DMA (measured: identical precision to an explicit rounding pass), which
    # skips all staging/cast work. numpy-side dtype is still float32.
    xT = nc.declare_dram_parameter("xT", [d, n], f32r, isOutput=False)
    wqT = nc.declare_dram_parameter("wqT", [d, f], f32r, isOutput=False)
    wkT = nc.declare_dram_parameter("wkT", [d, f], f32r, isOutput=False)
    wvT = nc.declare_dram_parameter("wvT", [d, f], f32r, isOutput=False)
    woT = nc.declare_dram_parameter("woT", [f, d], f32r, isOutput=False)
    # partial output is emitted TRANSPOSED [d, n]; the host sums + untransposes
    out = nc.declare_dram_parameter("out", [d, n], f32, isOutput=True)

    xT_c = xT.rearrange("(c p) n -> c p n", p=P)
    wqT_c = wqT.rearrange("(c p) f -> c p f", p=P)
    wkT_c = wkT.rearrange("(c p) f -> c p f", p=P)
    wvT_c = wvT.rearrange("(c p) f -> c p f", p=P)
    woT_c = woT.rearrange("(c p) n -> c p n", p=P)

    with tile.TileContext(nc) as tc:
        with (
            tc.tile_pool(name="qkv", bufs=1) as qkv,        # QT/KT/V residents
            tc.tile_pool(name="outT", bufs=1) as outp,
            tc.tile_pool(name="wo", bufs=1) as wop,
        ):
            QT_sb = qkv.tile([P, FC, n], f32r)
            # per-head K^T, zero-padded to a full 128-row stationary: head h
            # occupies partition rows po..po+hd (matching its rows in QT), the
            # other rows are zero.  K=64 matmuls run at 2 cyc/row on HW;
            # zero-padding to K=128 runs at 1 cyc/row for the same math.
            KTz_sb = qkv.tile([P, hpc, n], f32r)
            V_sb = qkv.tile([P, NT, hpc, hd + 1], f32r)
            outT_sb = outp.tile([P, FC, n], f32r)
            woT_sb = wop.tile([P, FC, d], f32r)
            # ones column of V_aug / zero fill of KTz: memset f32 consts, then
            # write via rounding DVE copies (direct memset on f32r fails
            # walrus codegen, and f32r matmul operands need rounding writers)
            ones_c = wop.tile([P, 1], f32)
            nc.vector.memset(ones_c[:], 1.0)
            nc.vector.tensor_copy(
                V_sb[:, :, :, hd : hd + 1],
                ones_c.to_broadcast([P, NT, hpc, 1]),
            )
            zero_c = wop.tile([P, 1], f32)
            nc.vector.memset(zero_c[:], 0.0)
            nc.vector.tensor_copy(
                KTz_sb[:], zero_c.to_broadcast([P, hpc, n])
            )

            # ---- Phase 0+1: load/convert inputs, projections ----
            # QT/KT run dc-OUTER holding all 8 PSUM banks, so the PE streams
            # right behind the xT DMA+cast pipeline instead of stalling on
            # full-tensor availability per accumulation group.
            with (
                tc.tile_pool(name="xw", bufs=1) as xw,
                tc.tile_pool(name="p1ps", bufs=4, space="PSUM") as p1ps,
            ):
                xT_r = xw.tile([P, DC, n], f32r)
                wqT_r = xw.tile([P, DC, f], f32r)
                wkT_r = xw.tile([P, DC, f], f32r)
                wvT_r = xw.tile([P, DC, f], f32r)

                # wq + xT interleaved per chunk: QT matmuls stream right
                # behind them; wk/wv/wo stream during QT/KT compute.
                for dc in range(DC):
                    nc.sync.dma_start(out=wqT_r[:, dc, :], in_=wqT_c[dc])
                    nc.sync.dma_start(out=xT_r[:, dc, :], in_=xT_c[dc])

                NG = n // QB  # 4 held accumulation groups per fc half-stage

                def proj_stage(w_sb, is_k):
                    # fc half-stages of 4 held banks: phase 2's scores pool
                    # gets the other 4 banks, so attention overlaps V compute
                    for fc in range(FC):
                        pss = [
                            p1ps.tile([P, QB], f32, tag="big", name=f"pj{g}")
                            for g in range(NG)
                        ]
                        for dc in range(DC):
                            for qc in range(NG):
                                nc.tensor.matmul(
                                    pss[qc][:],
                                    w_sb[:, dc, fc * P : (fc + 1) * P],
                                    xT_r[:, dc, qc * QB : (qc + 1) * QB],
                                    start=(dc == 0),
                                    stop=(dc == DC - 1),
                                )
                        for qc in range(NG):
                            sl = slice(qc * QB, (qc + 1) * QB)
                            if is_k:
                                # rows 0:64 = head 2fc (po=0), rows 64:128 =
                                # head 2fc+1 (po=64); keep row alignment
                                nc.vector.tensor_copy(
                                    KTz_sb[0:hd, 2 * fc, sl], pss[qc][0:hd, :]
                                )
                                nc.vector.tensor_copy(
                                    KTz_sb[hd : 2 * hd, 2 * fc + 1, sl],
                                    pss[qc][hd : 2 * hd, :],
                                )
                            else:
                                nc.vector.tensor_copy(
                                    QT_sb[:, fc, sl], pss[qc][:]
                                )

                proj_stage(wqT_r, False)
                for dc in range(DC):
                    nc.sync.dma_start(out=wkT_r[:, dc, :], in_=wkT_c[dc])
                proj_stage(wkT_r, True)
                for dc in range(DC):
                    nc.sync.dma_start(out=wvT_r[:, dc, :], in_=wvT_c[dc])
                for fc in range(FC):
                    nc.sync.dma_start(out=woT_sb[:, fc, :], in_=woT_c[fc])
                for nt in range(NT):
                    ps = p1ps.tile([P, QB], f32, tag="big")
                    for dc in range(DC):
                        nc.tensor.matmul(
                            ps[:, 0:f],
                            xT_r[:, dc, nt * P : (nt + 1) * P],
                            wvT_r[:, dc, :],
                            start=(dc == 0),
                            stop=(dc == DC - 1),
                        )
                    nc.vector.tensor_copy(
                        V_sb[:, nt, :, 0:hd],
                        ps[:, 0:f].rearrange("p (h e) -> p h e", h=hpc),
                    )

            # ---- Phase 2+3: attention passes, Wo folded in per q-half ----
            # Pass order is q-half-outer so each half's output projection can
            # run in the ACT-exp shadow of the next half. avo is copied to
            # SBUF immediately after accumulation so the single PSUM buffer
            # frees fast; normalize runs off the critical path from the copy.
            with (
                tc.tile_pool(name="scps", bufs=2, space="PSUM") as scps,
                tc.tile_pool(name="avps", bufs=1, space="PSUM") as avps,
                tc.tile_pool(name="wops", bufs=2, space="PSUM") as wopsp,
                tc.tile_pool(name="pt", bufs=3) as ptp,
                tc.tile_pool(name="norm", bufs=2) as normp,
                tc.tile_pool(name="wosb", bufs=4) as wosbp,
            ):
                for sh in range(NSC):
                    q0 = sh * SCW
                    for h in range(hpc):
                        fc = (h * hd) // P
                        po = (h * hd) % P
                        avo = avps.tile([hd + 1, SCW], f32, tag="avo")
                        for kc in range(NT):
                            sc = scps.tile([P, SCW], f32, tag="sc")
                            for qc in range(SCW // QB):
                                nc.tensor.matmul(
                                    sc[:, qc * QB : (qc + 1) * QB],
                                    KTz_sb[:, h, kc * P : (kc + 1) * P],
                                    QT_sb[
                                        :,
                                        fc,
                                        q0 + qc * QB : q0 + (qc + 1) * QB,
                                    ],
                                    start=True,
                                    stop=True,
                                )
                            pt = ptp.tile([P, SCW], f32r, tag="pt")
                            nc.scalar.activation(
                                pt[:], sc[:], mybir.ActivationFunctionType.Exp,
                                scale=scale,
                            )
                            for qc in range(SCW // QB):
                                nc.tensor.matmul(
                                    avo[:, qc * QB : (qc + 1) * QB],
                                    V_sb[:, kc, h, :],
                                    pt[:, qc * QB : (qc + 1) * QB],
                                    start=(kc == 0),
                                    stop=(kc == NT - 1),
                                )
                        # free avo fast, then normalize rows 0..hd-1 by row hd.
                        # reciprocal is single-lane-slow on a [1, SCW] row, so
                        # scatter the sums across partitions [128, SCW/128]
                        # via a small SBUF DMA round-trip first.
                        av_sb = normp.tile([hd + 1, SCW], f32, tag="av_sb")
                        nc.vector.tensor_copy(av_sb[:], avo[:])
                        rsh = normp.tile([P, SCW // P], f32, tag="rsh")
                        nc.sync.dma_start(out=rsh[:], in_=av_sb[hd : hd + 1, :])
                        rsh2 = normp.tile([P, SCW // P], f32, tag="rsh2")
                        nc.vector.reciprocal(rsh2[:], rsh[:])
                        recip = normp.tile([1, SCW], f32, tag="recip")
                        nc.sync.dma_start(out=recip[:], in_=rsh2[:])
                        bc = normp.tile([hd, SCW], f32, tag="bc")
                        nc.gpsimd.partition_broadcast(bc[:], recip[:])
                        nc.vector.tensor_mul(
                            outT_sb[po : po + hd, fc, q0 : q0 + SCW],
                            av_sb[0:hd, :],
                            bc[:],
                        )
                    # output projection for this q-half (woT stationary, 2
                    # moving q-blocks per weight load; emits partial^T [d, n])
                    for do in range(d // P):
                        pss = [
                            wopsp.tile([P, QB], f32, tag="wops", name=f"wo{i}")
                            for i in range(SCW // QB)
                        ]
                        for fc in range(FC):
                            for qc in range(SCW // QB):
                                nc.tensor.matmul(
                                    pss[qc][:],
                                    woT_sb[:, fc, do * P : (do + 1) * P],
                                    outT_sb[:, fc, q0 + qc * QB : q0 + (qc + 1) * QB],
                                    start=(fc == 0),
                                    stop=(fc == FC - 1),
                                )
                        for qc in range(SCW // QB):
                            ob = wosbp.tile([P, QB], f32, tag="ob")
                            nc.vector.tensor_copy(ob[:], pss[qc][:])
                            nc.sync.dma_start(
                                out=out[
                                    do * P : (do + 1) * P,
                                    q0 + qc * QB : q0 + (qc + 1) * QB,
                                ],
                                in_=ob[:],
                            )
    nc.finalize()
    return nc


def make_in_maps(x, Wq, Wk, Wv, Wo):
    """Shard full inputs into per-core DRAM parameter maps."""
    x = np.asarray(x, dtype=np.float32)
    Wq = np.asarray(Wq, dtype=np.float32)
    Wk = np.asarray(Wk, dtype=np.float32)
    Wv = np.asarray(Wv, dtype=np.float32)
    Wo = np.asarray(Wo, dtype=np.float32)
    xTs = [np.ascontiguousarray(x[b].T) for b in range(B)]
    WqT, WkT, WvT = Wq.T, Wk.T, Wv.T
    in_maps = []
    for c in range(N_CORES):
        b, g = c // (N_CORES // B), c % (N_CORES // B)
        fs = slice(g * F, (g + 1) * F)
        in_maps.append(
            {
                "xT": xTs[b],
                "wqT": np.ascontiguousarray(WqT[:, fs]),
                "wkT": np.ascontiguousarray(WkT[:, fs]),
                "wvT": np.ascontiguousarray(WvT[:, fs]),
                "woT": np.ascontiguousarray(Wo[:, fs].T),
            }
        )
    return in_maps


_NC_CACHE = {}


def _enable_ldw_opt():
    """Flip walrus --enable-ldw-opt to true: consecutive matmuls sharing a
    stationary operand skip the redundant LDWEIGHTS reload."""
    import concourse.bass_utils as bu

    if getattr(bu, "_ldw_opt_patched", False):
        return
    orig = bu.run_command

    def patched(argv, **kw):
        argv = [
            "--enable-ldw-opt=true" if a == "--enable-ldw-opt=false" else a
            for a in argv
        ]
        return orig(argv, **kw)

    bu.run_command = patched
    bu._ldw_opt_patched = True


def run(x, Wq, Wk, Wv, Wo, trace=False):
    from concourse.bass_utils import run_bass_kernel_spmd

    _enable_ldw_opt()

    if "nc" not in _NC_CACHE:
        _NC_CACHE["nc"] = build_nc()
    nc = _NC_CACHE["nc"]
    in_maps = make_in_maps(x, Wq, Wk, Wv, Wo)
    res = run_bass_kernel_spmd(nc, in_maps, core_ids=list(range(N_CORES)), trace=trace)
    parts = [np.asarray(res.results[i]["out"]) for i in range(N_CORES)]
    gpb = N_CORES // B
    # per-core partials are transposed [d, n]: sum the group, then untranspose
    full = np.stack(
        [
            sum(parts[b * gpb + 1 : (b + 1) * gpb], parts[b * gpb]).T
            for b in range(B)
        ]
    )
    return np.ascontiguousarray(full, dtype=np.float32), res


def kernel(x, Wq, bq, Wk, bk, Wv, bv, Wo, bo):
    full, _ = run(x, Wq, Wk, Wv, Wo)
    return full


# revision 38
# speedup vs baseline: 1.0054x; 1.0054x over previous
"""Multi-head attention kernel for 8 TRN2 NeuronCores.

Problem: b=2, n=2048, d=1024, heads=16, hd=64.
  q/k/v = x @ W{q,k,v}.T (+ zero bias)
  per head: softmax(q k^T / sqrt(d)) @ v
  out = concat @ Wo.T (+ zero bias)

Sharding (8 cores): data-parallel over batch (2) x tensor-parallel over
heads (16 heads -> 4 groups of 4). Core c handles batch c//4, heads
4*(c%4) .. 4*(c%4)+3 (feature slice of 256 columns). Wo is applied
row-parallel: each core emits a partial (n, d) output; the host sums the
4 partials per batch. No collectives needed.

All matmuls run in float32r (TF32-like: ~1.5e-4 rel err on a K=1024
contraction, 4x the fp32 rate). Operands feeding f32r matmuls must be
produced "rounded": DMA'd tensors get one DVE conversion pass; on-chip
tensors (Q^T/K^T/V/P^T/out^T) are written as f32r by their producing
copy/activation directly.

Per-core layouts (host pre-transposes so no on-device transposes at all):
  xT  (d, n)   : x[b].T
  wqT/wkT/wvT (d, 256), woT (256, d)
Pipeline:
  QT[feat, n], KT[feat, n]  (PE; contraction over d; f32r out via DVE)
  V[n, feat] + ones column  (PE; natural layout for AV stationary)
  per head h, k-chunk kc (128 k's), q-half sh (1024 q's):
     scores^T[128, 1024] = KT_h^T . QT_h   (PE, K=hd=64, psum)
     P^T = exp(scores^T / 32)              (ACT, psum->sbuf, f32r out)
     avo[65, q] += V_aug^T . P^T           (PE; row 64 = softmax sums)
  normalize: recip(sums) -> partition_broadcast -> mul  (DVE+GPSIMD)
  partial[n, d] = outT^T . woT (PE), DMA out via SBUF.

Biases are structurally zero in this problem spec and are skipped.
"""

import numpy as np

HEADS = 16
D = 1024
N = 2048
B = 2
N_CORES = 8
HPC = HEADS // (N_CORES // B)  # heads per core = 4
HD = D // HEADS                # 64
F = HPC * HD                   # 256 features per core
P = 128


def build_nc(n=N, d=D, hpc=HPC, hd=HD):
    """Build the per-core Bass program (SPMD: same program on all 8 cores)."""
    import concourse.bass as bass
    import concourse.tile as tile
    from concourse import bacc, mybir

    f32 = mybir.dt.float32
    f32r = mybir.dt.float32r
    f = hpc * hd            # per-core feature count (256)
    FC = f // P             # feature chunks (2)
    DC = d // P             # contraction chunks over d (8)
    NT = n // P             # n tiles / k chunks (16)
    QB = min(512, n)        # matmul moving block
    SCW = min(1024, n)      # scores psum width (2 banks)
    NSC = n // SCW          # q-halves
    scale = 1.0 / float(np.sqrt(np.float32(d)))

    nc = bacc.Bacc("TRN2")

    # inputs are declared float32r: the PE accepts raw fp32 bits via direct
    # DMA (measured: identical precision to an explicit rounding pass), which
    # skips all staging/cast work. numpy-side dtype is still float32.
    xT = nc.declare_dram_parameter("xT", [d, n], f32r, isOutput=False)
    wqT = nc.declare_dram_parameter("wqT", [d, f], f32r, isOutput=False)
    wkT = nc.declare_dram_parameter("wkT", [d, f], f32r, isOutput=False)
    wvT = nc.declare_dram_parameter("wvT", [d, f], f32r, isOutput=False)
    woT = nc.declare_dram_parameter("woT", [f, d], f32r, isOutput=False)
    # partial output is emitted TRANSPOSED [d, n]; the host sums + untransposes
    out = nc.declare_dram_parameter("out", [d, n], f32, isOutput=True)

    xT_c = xT.rearrange("(c p) n -> c p n", p=P)
    wqT_c = wqT.rearrange("(c p) f -> c p f", p=P)
    wkT_c = wkT.rearrange("(c p) f -> c p f", p=P)
    wvT_c = wvT.rearrange("(c p) f -> c p f", p=P)
    woT_c = woT.rearrange("(c p) n -> c p n", p=P)

    with tile.TileContext(nc) as tc:
        with (
            tc.tile_pool(name="qkv", bufs=1) as qkv,        # QT/KT/V residents
            tc.tile_pool(name="outT", bufs=1) as outp,
            tc.tile_pool(name="wo", bufs=1) as wop,
            # created BEFORE the phase-1 pools so their SBUF/PSUM ranges are
            # disjoint: early heads' scores/exp can overlap late projections
            tc.tile_pool(name="pt", bufs=3) as ptp,
            tc.tile_pool(name="scps", bufs=2, space="PSUM") as scps,
            tc.tile_pool(name="avps", bufs=1, space="PSUM") as avps,
        ):
            QT_sb = qkv.tile([P, FC, n], f32r)
            # per-head K^T, zero-padded to a full 128-row stationary: head h
            # occupies partition rows po..po+hd (matching its rows in QT), the
            # other rows are zero.  K=64 matmuls run at 2 cyc/row on HW;
            # zero-padding to K=128 runs at 1 cyc/row for the same math.
            KTz_sb = qkv.tile([P, hpc, n], f32r)
            V_sb = qkv.tile([P, NT, hpc, hd + 1], f32r)
            outT_sb = outp.tile([P, FC, n], f32r)
            woT_sb = wop.tile([P, FC, d], f32r)
            # ones column of V_aug / zero fill of KTz: memset f32 consts, then
            # write via rounding DVE copies (direct memset on f32r fails
            # walrus codegen, and f32r matmul operands need rounding writers)
            ones_c = wop.tile([P, 1], f32)
            nc.vector.memset(ones_c[:], 1.0)
            nc.vector.tensor_copy(
                V_sb[:, :, :, hd : hd + 1],
                ones_c.to_broadcast([P, NT, hpc, 1]),
            )
            zero_c = wop.tile([P, 1], f32)
            nc.vector.memset(zero_c[:], 0.0)
            nc.vector.tensor_copy(
                KTz_sb[:], zero_c.to_broadcast([P, hpc, n])
            )

            # ---- Phase 0+1: load/convert inputs, projections ----
            # QT/KT run dc-OUTER holding all 8 PSUM banks, so the PE streams
            # right behind the xT DMA+cast pipeline instead of stalling on
            # full-tensor availability per accumulation group.
            with (
                tc.tile_pool(name="xw", bufs=1) as xw,
                tc.tile_pool(name="p1ps", bufs=2, space="PSUM") as p1ps,
            ):
                xT_r = xw.tile([P, DC, n], f32r)
                wqT_r = xw.tile([P, DC, f], f32r)
                wkT_r = xw.tile([P, DC, f], f32r)
                wvT_r = xw.tile([P, DC, f], f32r)

                # wq + xT interleaved per chunk: QT matmuls stream right
                # behind them; wk/wv/wo stream during QT/KT compute.
                for dc in range(DC):
                    nc.sync.dma_start(out=wqT_r[:, dc, :], in_=wqT_c[dc])
                    nc.sync.dma_start(out=xT_r[:, dc, :], in_=xT_c[dc])

                def proj_fc(w_sb, is_k, fc):
                    # sub-stages of 2 held banks (p1ps bufs=2) so phase 2's
                    # avo/wo pools get disjoint PSUM and attention for the
                    # fc=0 heads overlaps the fc=1 projections
                    for qcp in range(0, n // QB, 2):
                        pss = [
                            p1ps.tile([P, QB], f32, tag="big", name=f"pj{g}")
                            for g in range(2)
                        ]
                        for dc in range(DC):
                            for j in range(2):
                                qc = qcp + j
                                nc.tensor.matmul(
                                    pss[j][:],
                                    w_sb[:, dc, fc * P : (fc + 1) * P],
                                    xT_r[:, dc, qc * QB : (qc + 1) * QB],
                                    start=(dc == 0),
                                    stop=(dc == DC - 1),
                                )
                        for j in range(2):
                            qc = qcp + j
                            sl = slice(qc * QB, (qc + 1) * QB)
                            if is_k:
                                # rows 0:64 = head 2fc (po=0), rows 64:128 =
                                # head 2fc+1 (po=64); keep row alignment
                                nc.vector.tensor_copy(
                                    KTz_sb[0:hd, 2 * fc, sl], pss[j][0:hd, :]
                                )
                                nc.vector.tensor_copy(
                                    KTz_sb[hd : 2 * hd, 2 * fc + 1, sl],
                                    pss[j][hd : 2 * hd, :],
                                )
                            else:
                                nc.vector.tensor_copy(
                                    QT_sb[:, fc, sl], pss[j][:]
                                )

                def v_stage():
                    for nt in range(NT):
                        ps = p1ps.tile([P, QB], f32, tag="big")
                        for dc in range(DC):
                            nc.tensor.matmul(
                                ps[:, 0:f],
                                xT_r[:, dc, nt * P : (nt + 1) * P],
                                wvT_r[:, dc, :],
                                start=(dc == 0),
                                stop=(dc == DC - 1),
                            )
                        nc.vector.tensor_copy(
                            V_sb[:, nt, :, 0:hd],
                            ps[:, 0:f].rearrange("p (h e) -> p h e", h=hpc),
                        )

                # fc=0 projections + V first: heads 0/1 attention can start
                # while the fc=1 projections still run on the PE
                proj_fc(wqT_r, False, 0)
                for dc in range(DC):
                    nc.sync.dma_start(out=wkT_r[:, dc, :], in_=wkT_c[dc])
                proj_fc(wkT_r, True, 0)
                for dc in range(DC):
                    nc.sync.dma_start(out=wvT_r[:, dc, :], in_=wvT_c[dc])
                v_stage()
                proj_fc(wqT_r, False, 1)
                proj_fc(wkT_r, True, 1)
                for fc in range(FC):
                    nc.sync.dma_start(out=woT_sb[:, fc, :], in_=woT_c[fc])

            # ---- Phase 2+3: attention passes, Wo folded in per q-half ----
            # Pass order is q-half-outer so each half's output projection can
            # run in the ACT-exp shadow of the next half. avo is copied to
            # SBUF immediately after accumulation so the single PSUM buffer
            # frees fast; normalize runs off the critical path from the copy.
            with (
                tc.tile_pool(name="wops", bufs=2, space="PSUM") as wopsp,
                tc.tile_pool(name="norm", bufs=2) as normp,
                tc.tile_pool(name="wosb", bufs=4) as wosbp,
            ):
                for sh in range(NSC):
                    q0 = sh * SCW
                    for h in range(hpc):
                        fc = (h * hd) // P
                        po = (h * hd) % P
                        avo = avps.tile([hd + 1, SCW], f32, tag="avo")
                        for kc in range(NT):
                            sc = scps.tile([P, SCW], f32, tag="sc")
                            for qc in range(SCW // QB):
                                nc.tensor.matmul(
                                    sc[:, qc * QB : (qc + 1) * QB],
                                    KTz_sb[:, h, kc * P : (kc + 1) * P],
                                    QT_sb[
                                        :,
                                        fc,
                                        q0 + qc * QB : q0 + (qc + 1) * QB,
                                    ],
                                    start=True,
                                    stop=True,
                                )
                            pt = ptp.tile([P, SCW], f32r, tag="pt")
                            nc.scalar.activation(
                                pt[:], sc[:], mybir.ActivationFunctionType.Exp,
                                scale=scale,
                            )
                            for qc in range(SCW // QB):
                                nc.tensor.matmul(
                                    avo[:, qc * QB : (qc + 1) * QB],
                                    V_sb[:, kc, h, :],
                                    pt[:, qc * QB : (qc + 1) * QB],
                                    start=(kc == 0),
                                    stop=(kc == NT - 1),
                                )
                        # free avo fast, then normalize rows 0..hd-1 by row hd.
                        # reciprocal is single-lane-slow on a [1, SCW] row, so
                        # scatter the sums across partitions [128, SCW/128]
                        # via a small SBUF DMA round-trip first.
                        av_sb = normp.tile([hd + 1, SCW], f32, tag="av_sb")
                        nc.vector.tensor_copy(av_sb[:], avo[:])
                        rsh = normp.tile([P, SCW // P], f32, tag="rsh")
                        nc.sync.dma_start(out=rsh[:], in_=av_sb[hd : hd + 1, :])
                        rsh2 = normp.tile([P, SCW // P], f32, tag="rsh2")
                        nc.vector.reciprocal(rsh2[:], rsh[:])
                        recip = normp.tile([1, SCW], f32, tag="recip")
                        nc.sync.dma_start(out=recip[:], in_=rsh2[:])
                        bc = normp.tile([hd, SCW], f32, tag="bc")
                        nc.gpsimd.partition_broadcast(bc[:], recip[:])
                        nc.vector.tensor_mul(
                            outT_sb[po : po + hd, fc, q0 : q0 + SCW],
                            av_sb[0:hd, :],
                            bc[:],
                        )
                    # output projection for this q-half (woT stationary, 2
                    # moving q-blocks per weight load; emits partial^T [d, n])
                    for do in range(d // P):
                        pss = [
                            wopsp.tile([P, QB], f32, tag="wops", name=f"wo{i}")
                            for i in range(SCW // QB)
                        ]
                        for fc in range(FC):
                            for qc in range(SCW // QB):
                                nc.tensor.matmul(
                                    pss[qc][:],
                                    woT_sb[:, fc, do * P : (do + 1) * P],
                                    outT_sb[:, fc, q0 + qc * QB : q0 + (qc + 1) * QB],
                                    start=(fc == 0),
                                    stop=(fc == FC - 1),
                                )
                        for qc in range(SCW // QB):
                            ob = wosbp.tile([P, QB], f32, tag="ob")
                            nc.vector.tensor_copy(ob[:], pss[qc][:])
                            nc.sync.dma_start(
                                out=out[
                                    do * P : (do + 1) * P,
                                    q0 + qc * QB : q0 + (qc + 1) * QB,
                                ],
                                in_=ob[:],
                            )
    nc.finalize()
    return nc


def make_in_maps(x, Wq, Wk, Wv, Wo):
    """Shard full inputs into per-core DRAM parameter maps."""
    x = np.asarray(x, dtype=np.float32)
    Wq = np.asarray(Wq, dtype=np.float32)
    Wk = np.asarray(Wk, dtype=np.float32)
    Wv = np.asarray(Wv, dtype=np.float32)
    Wo = np.asarray(Wo, dtype=np.float32)
    xTs = [np.ascontiguousarray(x[b].T) for b in range(B)]
    WqT, WkT, WvT = Wq.T, Wk.T, Wv.T
    in_maps = []
    for c in range(N_CORES):
        b, g = c // (N_CORES // B), c % (N_CORES // B)
        fs = slice(g * F, (g + 1) * F)
        in_maps.append(
            {
                "xT": xTs[b],
                "wqT": np.ascontiguousarray(WqT[:, fs]),
                "wkT": np.ascontiguousarray(WkT[:, fs]),
                "wvT": np.ascontiguousarray(WvT[:, fs]),
                "woT": np.ascontiguousarray(Wo[:, fs].T),
            }
        )
    return in_maps


_NC_CACHE = {}


def _enable_ldw_opt():
    """Flip walrus --enable-ldw-opt to true: consecutive matmuls sharing a
    stationary operand skip the redundant LDWEIGHTS reload."""
    import concourse.bass_utils as bu

    if getattr(bu, "_ldw_opt_patched", False):
        return
    orig = bu.run_command

    def patched(argv, **kw):
        argv = [
            "--enable-ldw-opt=true" if a == "--enable-ldw-opt=false" else a
            for a in argv
        ]
        return orig(argv, **kw)

    bu.run_command = patched
    bu._ldw_opt_patched = True


def run(x, Wq, Wk, Wv, Wo, trace=False):
    from concourse.bass_utils import run_bass_kernel_spmd

    _enable_ldw_opt()

    if "nc" not in _NC_CACHE:
        _NC_CACHE["nc"] = build_nc()
    nc = _NC_CACHE["nc"]
    in_maps = make_in_maps(x, Wq, Wk, Wv, Wo)
    res = run_bass_kernel_spmd(nc, in_maps, core_ids=list(range(N_CORES)), trace=trace)
    parts = [np.asarray(res.results[i]["out"]) for i in range(N_CORES)]
    gpb = N_CORES // B
    # per-core partials are transposed [d, n]: sum the group, then untranspose
    full = np.stack(
        [
            sum(parts[b * gpb + 1 : (b + 1) * gpb], parts[b * gpb]).T
            for b in range(B)
        ]
    )
    return np.ascontiguousarray(full, dtype=np.float32), res


def kernel(x, Wq, bq, Wk, bk, Wv, bv, Wo, bo):
    full, _ = run(x, Wq, Wk, Wv, Wo)
    return full


# revision 39
# speedup vs baseline: 1.0115x; 1.0061x over previous
"""Multi-head attention kernel for 8 TRN2 NeuronCores.

Problem: b=2, n=2048, d=1024, heads=16, hd=64.
  q/k/v = x @ W{q,k,v}.T (+ zero bias)
  per head: softmax(q k^T / sqrt(d)) @ v
  out = concat @ Wo.T (+ zero bias)

Sharding (8 cores): data-parallel over batch (2) x tensor-parallel over
heads (16 heads -> 4 groups of 4). Core c handles batch c//4, heads
4*(c%4) .. 4*(c%4)+3 (feature slice of 256 columns). Wo is applied
row-parallel: each core emits a partial output; the host sums the 4
partials per batch (and untransposes). No collectives needed.

All matmuls run in float32r (TF32-like: ~1.5e-4 rel err on a K=1024
contraction, 4x the fp32 PE rate, full rate only when the moving free
dim is >=256). Raw fp32 bits are DMA'd directly into f32r tiles
(measured identical to an explicit rounding pass). On-chip f32r
operands (Q^T/K^T/V/P^T/out^T) are written by rounding copy/activation
producers as the walrus verifier requires.

Key structure decisions (all measured on HW):
 - everything is pre-transposed on the host so the kernel needs zero
   on-device transposes: xT (d,n), wqT/wkT/wvT (d,256), woT (256,d).
 - Q^T/K^T [feat, n] via dc-outer accumulation streaming behind the
   xT DMA; V in natural [n, feat] layout with a ones column appended
   (the ones column accumulates the softmax denominators during AV).
 - K^T is stored zero-padded per head to a full 128-row stationary:
   K=64 matmuls run at 2 cyc/row and read as low PE activity (HAM
   clock-gates to half speed); zero-padded K=128 runs at 1 cyc/row.
 - scores^T[k, q] (PE) -> exp via ScalarE reading 2 PSUM banks per
   call (the ACT engine is the pacing floor: n*n*heads/core exps at 1
   elem/cycle/lane) -> AV accumulates V_aug^T . P^T in PSUM [65, q].
 - passes are (q-half, head)-ordered and their emission is interleaved
   with the fc=1 projections so the PE fills ACT-paced slack; each
   q-half's output projection runs in the next half's ACT shadow.
 - normalize: copy avo out of PSUM fast (frees the accumulator), then
   reciprocal in a [128, 8] partition-scattered layout (a [1, 1024]
   row reciprocal is single-lane and 60x slower), partition_broadcast
   on GpSimd, multiply on DVE.
 - output projection keeps woT stationary (2 moving blocks per weight
   load) and emits the partial TRANSPOSED [d, n]; the host untransposes.

Biases are structurally zero in this problem spec and are skipped.
"""

import numpy as np

HEADS = 16
D = 1024
N = 2048
B = 2
N_CORES = 8
HPC = HEADS // (N_CORES // B)  # heads per core = 4
HD = D // HEADS                # 64
F = HPC * HD                   # 256 features per core
P = 128


def build_nc(n=N, d=D, hpc=HPC, hd=HD):
    """Build the per-core Bass program (SPMD: same program on all 8 cores)."""
    import concourse.bass as bass
    import concourse.tile as tile
    from concourse import bacc, mybir

    f32 = mybir.dt.float32
    f32r = mybir.dt.float32r
    f = hpc * hd            # per-core feature count (256)
    FC = f // P             # feature chunks (2)
    DC = d // P             # contraction chunks over d (8)
    NT = n // P             # n tiles / k chunks (16)
    QB = min(512, n)        # matmul moving block
    SCW = min(1024, n)      # scores psum width (2 banks)
    NSC = n // SCW          # q-halves
    scale = 1.0 / float(np.sqrt(np.float32(d)))

    nc = bacc.Bacc("TRN2")

    xT = nc.declare_dram_parameter("xT", [d, n], f32r, isOutput=False)
    wqT = nc.declare_dram_parameter("wqT", [d, f], f32r, isOutput=False)
    wkT = nc.declare_dram_parameter("wkT", [d, f], f32r, isOutput=False)
    wvT = nc.declare_dram_parameter("wvT", [d, f], f32r, isOutput=False)
    woT = nc.declare_dram_parameter("woT", [f, d], f32r, isOutput=False)
    out = nc.declare_dram_parameter("out", [d, n], f32, isOutput=True)

    xT_c = xT.rearrange("(c p) n -> c p n", p=P)
    wqT_c = wqT.rearrange("(c p) f -> c p f", p=P)
    wkT_c = wkT.rearrange("(c p) f -> c p f", p=P)
    wvT_c = wvT.rearrange("(c p) f -> c p f", p=P)
    woT_c = woT.rearrange("(c p) n -> c p n", p=P)

    with tile.TileContext(nc) as tc:
        with (
            tc.tile_pool(name="qkv", bufs=1) as qkv,
            tc.tile_pool(name="outT", bufs=1) as outp,
            # phase-2 pools created before the phase-1 pools so their
            # SBUF/PSUM ranges are disjoint: early heads' attention overlaps
            # the fc=1 projections with no pool-reuse serialization
            tc.tile_pool(name="pt", bufs=2) as ptp,
            tc.tile_pool(name="norm", bufs=1) as normp,
            tc.tile_pool(name="scps", bufs=2, space="PSUM") as scps,
            tc.tile_pool(name="avps", bufs=1, space="PSUM") as avps,
        ):
            QT_sb = qkv.tile([P, FC, n], f32r)
            # per-head K^T, zero-padded to a full 128-row stationary (head h
            # occupies partition rows po..po+hd, matching its rows in QT)
            KTz_sb = qkv.tile([P, hpc, n], f32r)
            V_sb = qkv.tile([P, NT, hpc, hd + 1], f32r)
            outT_sb = outp.tile([P, FC, n], f32r)
            # ones column of V_aug / zero fill of KTz: memset f32 consts, then
            # write via rounding DVE copies (direct memset on f32r fails
            # walrus codegen, and f32r matmul operands need rounding writers)
            ones_c = outp.tile([P, 1], f32)
            nc.vector.memset(ones_c[:], 1.0)
            nc.vector.tensor_copy(
                V_sb[:, :, :, hd : hd + 1],
                ones_c.to_broadcast([P, NT, hpc, 1]),
            )
            zero_c = outp.tile([P, 1], f32)
            nc.vector.memset(zero_c[:], 0.0)
            nc.vector.tensor_copy(
                KTz_sb[:], zero_c.to_broadcast([P, hpc, n])
            )

            def do_pass(h, sh):
                """Attention for (head h, q-half sh): scores^T -> exp -> AV
                accumulate -> normalize into outT_sb."""
                fc = (h * hd) // P
                po = (h * hd) % P
                q0 = sh * SCW
                avo = avps.tile([hd + 1, SCW], f32, tag="avo")
                for kc in range(NT):
                    sc = scps.tile([P, SCW], f32, tag="sc")
                    for qc in range(SCW // QB):
                        nc.tensor.matmul(
                            sc[:, qc * QB : (qc + 1) * QB],
                            KTz_sb[:, h, kc * P : (kc + 1) * P],
                            QT_sb[:, fc, q0 + qc * QB : q0 + (qc + 1) * QB],
                            start=True,
                            stop=True,
                        )
                    pt = ptp.tile([P, SCW], f32r, tag="pt")
                    nc.scalar.activation(
                        pt[:], sc[:], mybir.ActivationFunctionType.Exp,
                        scale=scale,
                    )
                    for qc in range(SCW // QB):
                        nc.tensor.matmul(
                            avo[:, qc * QB : (qc + 1) * QB],
                            V_sb[:, kc, h, :],
                            pt[:, qc * QB : (qc + 1) * QB],
                            start=(kc == 0),
                            stop=(kc == NT - 1),
                        )
                # free avo fast, then normalize rows 0..hd-1 by row hd (the
                # softmax sums). reciprocal is single-lane-slow on a [1, SCW]
                # row, so scatter the sums across partitions via a small
                # SBUF DMA round-trip first.
                av_sb = normp.tile([hd + 1, SCW], f32, tag="av_sb")
                nc.vector.tensor_copy(av_sb[:], avo[:])
                rsh = normp.tile([P, SCW // P], f32, tag="rsh")
                nc.sync.dma_start(out=rsh[:], in_=av_sb[hd : hd + 1, :])
                rsh2 = normp.tile([P, SCW // P], f32, tag="rsh2")
                nc.vector.reciprocal(rsh2[:], rsh[:])
                recip = normp.tile([1, SCW], f32, tag="recip")
                nc.sync.dma_start(out=recip[:], in_=rsh2[:])
                bc = normp.tile([hd, SCW], f32, tag="bc")
                nc.gpsimd.partition_broadcast(bc[:], recip[:])
                nc.vector.tensor_mul(
                    outT_sb[po : po + hd, fc, q0 : q0 + SCW],
                    av_sb[0:hd, :],
                    bc[:],
                )

            # ---- Phase 1 + first q-half heads 0/1, emission-interleaved ----
            with (
                tc.tile_pool(name="xw", bufs=1) as xw,
                tc.tile_pool(name="p1ps", bufs=2, space="PSUM") as p1ps,
            ):
                xT_r = xw.tile([P, DC, n], f32r)
                wqT_r = xw.tile([P, DC, f], f32r)
                wkT_r = xw.tile([P, DC, f], f32r)
                wvT_r = xw.tile([P, DC, f], f32r)

                # wq + xT interleaved per chunk: QT matmuls stream right
                # behind them; wk/wv stream during QT/KT compute.
                for dc in range(DC):
                    nc.sync.dma_start(out=wqT_r[:, dc, :], in_=wqT_c[dc])
                    nc.sync.dma_start(out=xT_r[:, dc, :], in_=xT_c[dc])

                def proj_fc(w_sb, is_k, fc):
                    # dc-outer accumulation in sub-stages of 2 held banks
                    for qcp in range(0, n // QB, 2):
                        pss = [
                            p1ps.tile([P, QB], f32, tag="big", name=f"pj{g}")
                            for g in range(2)
                        ]
                        for dc in range(DC):
                            for j in range(2):
                                qc = qcp + j
                                nc.tensor.matmul(
                                    pss[j][:],
                                    w_sb[:, dc, fc * P : (fc + 1) * P],
                                    xT_r[:, dc, qc * QB : (qc + 1) * QB],
                                    start=(dc == 0),
                                    stop=(dc == DC - 1),
                                )
                        for j in range(2):
                            qc = qcp + j
                            sl = slice(qc * QB, (qc + 1) * QB)
                            if is_k:
                                # rows 0:64 = head 2fc (po=0), rows 64:128 =
                                # head 2fc+1 (po=64); keep row alignment
                                nc.vector.tensor_copy(
                                    KTz_sb[0:hd, 2 * fc, sl], pss[j][0:hd, :]
                                )
                                nc.vector.tensor_copy(
                                    KTz_sb[hd : 2 * hd, 2 * fc + 1, sl],
                                    pss[j][hd : 2 * hd, :],
                                )
                            else:
                                nc.vector.tensor_copy(
                                    QT_sb[:, fc, sl], pss[j][:]
                                )

                def v_stage():
                    for nt in range(NT):
                        ps = p1ps.tile([P, QB], f32, tag="big")
                        for dc in range(DC):
                            nc.tensor.matmul(
                                ps[:, 0:f],
                                xT_r[:, dc, nt * P : (nt + 1) * P],
                                wvT_r[:, dc, :],
                                start=(dc == 0),
                                stop=(dc == DC - 1),
                            )
                        nc.vector.tensor_copy(
                            V_sb[:, nt, :, 0:hd],
                            ps[:, 0:f].rearrange("p (h e) -> p h e", h=hpc),
                        )

                proj_fc(wqT_r, False, 0)
                for dc in range(DC):
                    nc.sync.dma_start(out=wkT_r[:, dc, :], in_=wkT_c[dc])
                proj_fc(wkT_r, True, 0)
                for dc in range(DC):
                    nc.sync.dma_start(out=wvT_r[:, dc, :], in_=wvT_c[dc])
                v_stage()
                # heads 0/1 of the first q-half interleave with the fc=1
                # projections: the PE fills the ACT-paced slack
                do_pass(0, 0)
                proj_fc(wqT_r, False, 1)
                do_pass(1, 0)
                proj_fc(wkT_r, True, 1)

            # ---- remaining passes + per-q-half output projection ----
            with (
                tc.tile_pool(name="wo", bufs=1) as wop,
                tc.tile_pool(name="wops", bufs=2, space="PSUM") as wopsp,
                tc.tile_pool(name="wosb", bufs=4) as wosbp,
            ):
                woT_sb = wop.tile([P, FC, d], f32r)
                for fc in range(FC):
                    nc.sync.dma_start(out=woT_sb[:, fc, :], in_=woT_c[fc])

                def wo_half(sh):
                    # output projection for q-half sh (woT stationary, 2
                    # moving q-blocks per weight load; emits partial^T [d, n])
                    q0 = sh * SCW
                    for do in range(d // P):
                        pss = [
                            wopsp.tile([P, QB], f32, tag="wops", name=f"wo{i}")
                            for i in range(SCW // QB)
                        ]
                        for fc in range(FC):
                            for qc in range(SCW // QB):
                                nc.tensor.matmul(
                                    pss[qc][:],
                                    woT_sb[:, fc, do * P : (do + 1) * P],
                                    outT_sb[
                                        :, fc, q0 + qc * QB : q0 + (qc + 1) * QB
                                    ],
                                    start=(fc == 0),
                                    stop=(fc == FC - 1),
                                )
                        for qc in range(SCW // QB):
                            ob = wosbp.tile([P, QB], f32, tag="ob")
                            nc.vector.tensor_copy(ob[:], pss[qc][:])
                            nc.sync.dma_start(
                                out=out[
                                    do * P : (do + 1) * P,
                                    q0 + qc * QB : q0 + (qc + 1) * QB,
                                ],
                                in_=ob[:],
                            )

                do_pass(2, 0)
                do_pass(3, 0)
                wo_half(0)
                for h in range(hpc):
                    do_pass(h, 1)
                wo_half(1)
    nc.finalize()
    return nc


def make_in_maps(x, Wq, Wk, Wv, Wo):
    """Shard full inputs into per-core DRAM parameter maps."""
    x = np.asarray(x, dtype=np.float32)
    Wq = np.asarray(Wq, dtype=np.float32)
    Wk = np.asarray(Wk, dtype=np.float32)
    Wv = np.asarray(Wv, dtype=np.float32)
    Wo = np.asarray(Wo, dtype=np.float32)
    xTs = [np.ascontiguousarray(x[b].T) for b in range(B)]
    WqT, WkT, WvT = Wq.T, Wk.T, Wv.T
    in_maps = []
    for c in range(N_CORES):
        b, g = c // (N_CORES // B), c % (N_CORES // B)
        fs = slice(g * F, (g + 1) * F)
        in_maps.append(
            {
                "xT": xTs[b],
                "wqT": np.ascontiguousarray(WqT[:, fs]),
                "wkT": np.ascontiguousarray(WkT[:, fs]),
                "wvT": np.ascontiguousarray(WvT[:, fs]),
                "woT": np.ascontiguousarray(Wo[:, fs].T),
            }
        )
    return in_maps


_NC_CACHE = {}


def _enable_ldw_opt():
    """Flip walrus --enable-ldw-opt to true: consecutive matmuls sharing a
    stationary operand skip the redundant LDWEIGHTS reload."""
    import concourse.bass_utils as bu

    if getattr(bu, "_ldw_opt_patched", False):
        return
    orig = bu.run_command

    def patched(argv, **kw):
        argv = [
            "--enable-ldw-opt=true" if a == "--enable-ldw-opt=false" else a
            for a in argv
        ]
        return orig(argv, **kw)

    bu.run_command = patched
    bu._ldw_opt_patched = True


def run(x, Wq, Wk, Wv, Wo, trace=False):
    from concourse.bass_utils import run_bass_kernel_spmd

    _enable_ldw_opt()
    if "nc" not in _NC_CACHE:
        _NC_CACHE["nc"] = build_nc()
    nc = _NC_CACHE["nc"]
    in_maps = make_in_maps(x, Wq, Wk, Wv, Wo)
    res = run_bass_kernel_spmd(nc, in_maps, core_ids=list(range(N_CORES)), trace=trace)
    parts = [np.asarray(res.results[i]["out"]) for i in range(N_CORES)]
    gpb = N_CORES // B
    # per-core partials are transposed [d, n]: sum the group, then untranspose
    full = np.stack(
        [
            sum(parts[b * gpb + 1 : (b + 1) * gpb], parts[b * gpb]).T
            for b in range(B)
        ]
    )
    return np.ascontiguousarray(full, dtype=np.float32), res


def kernel(x, Wq, bq, Wk, bk, Wv, bv, Wo, bo):
    full, _ = run(x, Wq, Wk, Wv, Wo)
    return full


# revision 42
# speedup vs baseline: 1.0444x; 1.0325x over previous
"""Multi-head attention kernel for 8 TRN2 NeuronCores.

Problem: b=2, n=2048, d=1024, heads=16, hd=64.
  q/k/v = x @ W{q,k,v}.T (+ zero bias)
  per head: softmax(q k^T / sqrt(d)) @ v
  out = concat @ Wo.T (+ zero bias)

Sharding (8 cores): data-parallel over batch (2) x tensor-parallel over
heads (16 heads -> 4 groups of 4). Core c handles batch c//4, heads
4*(c%4) .. 4*(c%4)+3 (feature slice of 256 columns). Wo is applied
row-parallel: each core emits a partial output; the host sums the 4
partials per batch (and untransposes). No collectives needed.

All matmuls run in float32r (TF32-like: ~1.5e-4 rel err on a K=1024
contraction, 4x the fp32 PE rate, full rate only when the moving free
dim is >=256). Raw fp32 bits are DMA'd directly into f32r tiles
(measured identical to an explicit rounding pass). On-chip f32r
operands (Q^T/K^T/V/P^T/out^T) are written by rounding copy/activation
producers as the walrus verifier requires.

Key structure decisions (all measured on HW):
 - everything is pre-transposed on the host so the kernel needs zero
   on-device transposes: xT (d,n), wqT/wkT/wvT (d,256), woT (256,d).
 - Q^T/K^T [feat, n] via dc-outer accumulation streaming behind the
   xT DMA; V in natural [n, feat] layout with a ones column appended
   (the ones column accumulates the softmax denominators during AV).
 - K^T is stored zero-padded per head to a full 128-row stationary:
   K=64 matmuls run at 2 cyc/row and read as low PE activity (HAM
   clock-gates to half speed); zero-padded K=128 runs at 1 cyc/row.
 - scores^T[k, q] (PE) -> exp via ScalarE reading 2 PSUM banks per
   call (the ACT engine is the pacing floor: n*n*heads/core exps at 1
   elem/cycle/lane) -> AV accumulates V_aug^T . P^T in PSUM [65, q].
 - passes are (q-half, head)-ordered and their emission is interleaved
   with the fc=1 projections so the PE fills ACT-paced slack; each
   q-half's output projection runs in the next half's ACT shadow.
 - normalize: copy avo out of PSUM fast (frees the accumulator), then
   reciprocal in a [128, 8] partition-scattered layout (a [1, 1024]
   row reciprocal is single-lane and 60x slower), partition_broadcast
   on GpSimd, multiply on DVE.
 - output projection keeps woT stationary (2 moving blocks per weight
   load) and emits the partial TRANSPOSED [d, n]; the host untransposes.

Biases are structurally zero in this problem spec and are skipped.
"""

import numpy as np

HEADS = 16
D = 1024
N = 2048
B = 2
N_CORES = 8
HPC = HEADS // (N_CORES // B)  # heads per core = 4
HD = D // HEADS                # 64
F = HPC * HD                   # 256 features per core
P = 128


def build_nc(n=N, d=D, hpc=HPC, hd=HD):
    """Build the per-core Bass program (SPMD: same program on all 8 cores)."""
    import concourse.bass as bass
    import concourse.tile as tile
    from concourse import bacc, mybir

    f32 = mybir.dt.float32
    f32r = mybir.dt.float32r
    f = hpc * hd            # per-core feature count (256)
    FC = f // P             # feature chunks (2)
    DC = d // P             # contraction chunks over d (8)
    NT = n // P             # n tiles / k chunks (16)
    QB = min(512, n)        # matmul moving block
    SCW = min(1024, n)      # scores psum width (2 banks)
    NSC = n // SCW          # q-halves
    scale = 1.0 / float(np.sqrt(np.float32(d)))

    nc = bacc.Bacc("TRN2")

    xT = nc.declare_dram_parameter("xT", [d, n], f32r, isOutput=False)
    wqT = nc.declare_dram_parameter("wqT", [d, f], f32r, isOutput=False)
    wkT = nc.declare_dram_parameter("wkT", [d, f], f32r, isOutput=False)
    wvT = nc.declare_dram_parameter("wvT", [d, f], f32r, isOutput=False)
    woT = nc.declare_dram_parameter("woT", [f, d], f32r, isOutput=False)
    out = nc.declare_dram_parameter("out", [d, n], f32, isOutput=True)

    xT_c = xT.rearrange("(c p) n -> c p n", p=P)
    wqT_c = wqT.rearrange("(c p) f -> c p f", p=P)
    wkT_c = wkT.rearrange("(c p) f -> c p f", p=P)
    wvT_c = wvT.rearrange("(c p) f -> c p f", p=P)
    woT_c = woT.rearrange("(c p) n -> c p n", p=P)

    with tile.TileContext(nc) as tc:
        with (
            tc.tile_pool(name="qkv", bufs=1) as qkv,
            tc.tile_pool(name="outT", bufs=1) as outp,
            # phase-2 pools created before the phase-1 pools so their
            # SBUF/PSUM ranges are disjoint: early heads' attention overlaps
            # the fc=1 projections with no pool-reuse serialization
            tc.tile_pool(name="pt", bufs=2) as ptp,
            tc.tile_pool(name="norm", bufs=1) as normp,
            tc.tile_pool(name="scps", bufs=2, space="PSUM") as scps,
            tc.tile_pool(name="avps", bufs=1, space="PSUM") as avps,
        ):
            QT_sb = qkv.tile([P, FC, n], f32r)
            # per-head K^T, zero-padded to a full 128-row stationary (head h
            # occupies partition rows po..po+hd, matching its rows in QT)
            KTz_sb = qkv.tile([P, hpc, n], f32r)
            V_sb = qkv.tile([P, NT, hpc, hd + 1], f32r)
            outT_sb = outp.tile([P, FC, n], f32r)
            # ones column of V_aug / zero fill of KTz: memset f32 consts, then
            # write via rounding DVE copies (direct memset on f32r fails
            # walrus codegen, and f32r matmul operands need rounding writers)
            ones_c = outp.tile([P, 1], f32)
            nc.vector.memset(ones_c[:], 1.0)
            nc.vector.tensor_copy(
                V_sb[:, :, :, hd : hd + 1],
                ones_c.to_broadcast([P, NT, hpc, 1]),
            )
            zero_c = outp.tile([P, 1], f32)
            nc.vector.memset(zero_c[:], 0.0)
            nc.vector.tensor_copy(
                KTz_sb[:], zero_c.to_broadcast([P, hpc, n])
            )

            def do_pass(h, sh, pre_kc=None):
                """Attention for (head h, q-half sh): scores^T -> exp -> AV
                accumulate -> normalize into outT_sb."""
                fc = (h * hd) // P
                po = (h * hd) % P
                q0 = sh * SCW
                avo = avps.tile([hd + 1, SCW], f32, tag="avo")
                for kc in range(NT):
                    if pre_kc is not None:
                        pre_kc(kc)
                    sc = scps.tile([P, SCW], f32, tag="sc")
                    for qc in range(SCW // QB):
                        nc.tensor.matmul(
                            sc[:, qc * QB : (qc + 1) * QB],
                            KTz_sb[:, h, kc * P : (kc + 1) * P],
                            QT_sb[:, fc, q0 + qc * QB : q0 + (qc + 1) * QB],
                            start=True,
                            stop=True,
                        )
                    pt = ptp.tile([P, SCW], f32r, tag="pt")
                    nc.scalar.activation(
                        pt[:], sc[:], mybir.ActivationFunctionType.Exp,
                        scale=scale,
                    )
                    for qc in range(SCW // QB):
                        nc.tensor.matmul(
                            avo[:, qc * QB : (qc + 1) * QB],
                            V_sb[:, kc, h, :],
                            pt[:, qc * QB : (qc + 1) * QB],
                            start=(kc == 0),
                            stop=(kc == NT - 1),
                        )
                # free avo fast, then normalize rows 0..hd-1 by row hd (the
                # softmax sums). reciprocal is single-lane-slow on a [1, SCW]
                # row, so scatter the sums across partitions via a small
                # SBUF DMA round-trip first.
                av_sb = normp.tile([hd + 1, SCW], f32, tag="av_sb")
                nc.vector.tensor_copy(av_sb[:], avo[:])
                rsh = normp.tile([P, SCW // P], f32, tag="rsh")
                nc.sync.dma_start(out=rsh[:], in_=av_sb[hd : hd + 1, :])
                rsh2 = normp.tile([P, SCW // P], f32, tag="rsh2")
                nc.vector.reciprocal(rsh2[:], rsh[:])
                recip = normp.tile([1, SCW], f32, tag="recip")
                nc.sync.dma_start(out=recip[:], in_=rsh2[:])
                bc = normp.tile([hd, SCW], f32, tag="bc")
                nc.gpsimd.partition_broadcast(bc[:], recip[:])
                nc.vector.tensor_mul(
                    outT_sb[po : po + hd, fc, q0 : q0 + SCW],
                    av_sb[0:hd, :],
                    bc[:],
                )

            # ---- Phase 1 + first q-half heads 0/1, emission-interleaved ----
            with (
                tc.tile_pool(name="xw", bufs=1) as xw,
                tc.tile_pool(name="p1ps", bufs=2, space="PSUM") as p1ps,
            ):
                xT_r = xw.tile([P, DC, n], f32r)
                wqT_r = xw.tile([P, DC, f], f32r)
                wkT_r = xw.tile([P, DC, f], f32r)
                wvT_r = xw.tile([P, DC, f], f32r)

                # wq + xT interleaved per chunk: QT matmuls stream right
                # behind them; wk/wv stream during QT/KT compute.
                for dc in range(DC):
                    nc.sync.dma_start(out=wqT_r[:, dc, :], in_=wqT_c[dc])
                    nc.sync.dma_start(out=xT_r[:, dc, :], in_=xT_c[dc])

                def proj_fc(w_sb, is_k, fc):
                    # dc-outer accumulation in sub-stages of 2 held banks
                    for qcp in range(0, n // QB, 2):
                        pss = [
                            p1ps.tile([P, QB], f32, tag="big", name=f"pj{g}")
                            for g in range(2)
                        ]
                        for dc in range(DC):
                            for j in range(2):
                                qc = qcp + j
                                nc.tensor.matmul(
                                    pss[j][:],
                                    w_sb[:, dc, fc * P : (fc + 1) * P],
                                    xT_r[:, dc, qc * QB : (qc + 1) * QB],
                                    start=(dc == 0),
                                    stop=(dc == DC - 1),
                                )
                        for j in range(2):
                            qc = qcp + j
                            sl = slice(qc * QB, (qc + 1) * QB)
                            if is_k:
                                # rows 0:64 = head 2fc (po=0), rows 64:128 =
                                # head 2fc+1 (po=64); keep row alignment
                                nc.vector.tensor_copy(
                                    KTz_sb[0:hd, 2 * fc, sl], pss[j][0:hd, :]
                                )
                                nc.vector.tensor_copy(
                                    KTz_sb[hd : 2 * hd, 2 * fc + 1, sl],
                                    pss[j][hd : 2 * hd, :],
                                )
                            else:
                                nc.vector.tensor_copy(
                                    QT_sb[:, fc, sl], pss[j][:]
                                )

                def v_tile(nt):
                    ps = p1ps.tile([P, QB], f32, tag="big", name="vps")
                    for dc in range(DC):
                        nc.tensor.matmul(
                            ps[:, 0:f],
                            xT_r[:, dc, nt * P : (nt + 1) * P],
                            wvT_r[:, dc, :],
                            start=(dc == 0),
                            stop=(dc == DC - 1),
                        )
                    nc.vector.tensor_copy(
                        V_sb[:, nt, :, 0:hd],
                        ps[:, 0:f].rearrange("p (h e) -> p h e", h=hpc),
                    )

                proj_fc(wqT_r, False, 0)
                for dc in range(DC):
                    nc.sync.dma_start(out=wkT_r[:, dc, :], in_=wkT_c[dc])
                proj_fc(wkT_r, True, 0)
                for dc in range(DC):
                    nc.sync.dma_start(out=wvT_r[:, dc, :], in_=wvT_c[dc])
                # head 0's pass interleaves the V tiles it consumes (V[kc] is
                # exactly what AV(kc) needs); head 1's pass precedes the fc=1
                # projections in emission order so the scores feeding the ACT
                # engine win scheduling priority and the projections fill the
                # PE's ACT-paced slack.
                do_pass(0, 0, pre_kc=v_tile)
                do_pass(1, 0)
                proj_fc(wqT_r, False, 1)
                proj_fc(wkT_r, True, 1)

            # ---- remaining passes + per-q-half output projection ----
            with (
                tc.tile_pool(name="wo", bufs=1) as wop,
                tc.tile_pool(name="wops", bufs=2, space="PSUM") as wopsp,
                tc.tile_pool(name="wosb", bufs=4) as wosbp,
            ):
                woT_sb = wop.tile([P, FC, d], f32r)
                for fc in range(FC):
                    nc.sync.dma_start(out=woT_sb[:, fc, :], in_=woT_c[fc])

                def wo_half(sh):
                    # output projection for q-half sh (woT stationary, 2
                    # moving q-blocks per weight load; emits partial^T [d, n])
                    q0 = sh * SCW
                    for do in range(d // P):
                        pss = [
                            wopsp.tile([P, QB], f32, tag="wops", name=f"wo{i}")
                            for i in range(SCW // QB)
                        ]
                        for fc in range(FC):
                            for qc in range(SCW // QB):
                                nc.tensor.matmul(
                                    pss[qc][:],
                                    woT_sb[:, fc, do * P : (do + 1) * P],
                                    outT_sb[
                                        :, fc, q0 + qc * QB : q0 + (qc + 1) * QB
                                    ],
                                    start=(fc == 0),
                                    stop=(fc == FC - 1),
                                )
                        for qc in range(SCW // QB):
                            ob = wosbp.tile([P, QB], f32, tag="ob")
                            nc.vector.tensor_copy(ob[:], pss[qc][:])
                            nc.sync.dma_start(
                                out=out[
                                    do * P : (do + 1) * P,
                                    q0 + qc * QB : q0 + (qc + 1) * QB,
                                ],
                                in_=ob[:],
                            )

                do_pass(2, 0)
                do_pass(3, 0)
                do_pass(0, 1)
                do_pass(1, 1)
                wo_half(0)
                do_pass(2, 1)
                do_pass(3, 1)
                wo_half(1)
    nc.finalize()
    return nc


def make_in_maps(x, Wq, Wk, Wv, Wo):
    """Shard full inputs into per-core DRAM parameter maps."""
    x = np.asarray(x, dtype=np.float32)
    Wq = np.asarray(Wq, dtype=np.float32)
    Wk = np.asarray(Wk, dtype=np.float32)
    Wv = np.asarray(Wv, dtype=np.float32)
    Wo = np.asarray(Wo, dtype=np.float32)
    xTs = [np.ascontiguousarray(x[b].T) for b in range(B)]
    WqT, WkT, WvT = Wq.T, Wk.T, Wv.T
    in_maps = []
    for c in range(N_CORES):
        b, g = c // (N_CORES // B), c % (N_CORES // B)
        fs = slice(g * F, (g + 1) * F)
        in_maps.append(
            {
                "xT": xTs[b],
                "wqT": np.ascontiguousarray(WqT[:, fs]),
                "wkT": np.ascontiguousarray(WkT[:, fs]),
                "wvT": np.ascontiguousarray(WvT[:, fs]),
                "woT": np.ascontiguousarray(Wo[:, fs].T),
            }
        )
    return in_maps


_NC_CACHE = {}


def _enable_ldw_opt():
    """Flip walrus --enable-ldw-opt to true: consecutive matmuls sharing a
    stationary operand skip the redundant LDWEIGHTS reload."""
    import concourse.bass_utils as bu

    if getattr(bu, "_ldw_opt_patched", False):
        return
    orig = bu.run_command

    def patched(argv, **kw):
        argv = [
            "--enable-ldw-opt=true" if a == "--enable-ldw-opt=false" else a
            for a in argv
        ]
        return orig(argv, **kw)

    bu.run_command = patched
    bu._ldw_opt_patched = True


def run(x, Wq, Wk, Wv, Wo, trace=False):
    from concourse.bass_utils import run_bass_kernel_spmd

    _enable_ldw_opt()
    if "nc" not in _NC_CACHE:
        _NC_CACHE["nc"] = build_nc()
    nc = _NC_CACHE["nc"]
    in_maps = make_in_maps(x, Wq, Wk, Wv, Wo)
    res = run_bass_kernel_spmd(nc, in_maps, core_ids=list(range(N_CORES)), trace=trace)
    parts = [np.asarray(res.results[i]["out"]) for i in range(N_CORES)]
    gpb = N_CORES // B
    # per-core partials are transposed [d, n]: sum the group, then untranspose
    full = np.stack(
        [
            sum(parts[b * gpb + 1 : (b + 1) * gpb], parts[b * gpb]).T
            for b in range(B)
        ]
    )
    return np.ascontiguousarray(full, dtype=np.float32), res


def kernel(x, Wq, bq, Wk, bk, Wv, bv, Wo, bo):
    full, _ = run(x, Wq, Wk, Wv, Wo)
    return full
